# revision 16
# baseline (speedup 1.0000x reference)
"""Trainium2 Bass kernel for nn_Block_17978733101066.

ConvNeXt-style block: channels-first LayerNorm -> NNMF conv (25 multiplicative
updates with grouped 3x3 convs) residual branch, then channels-last LayerNorm +
MLP residual branch.  Input x: (8, 96, 56, 56) f32.

Strategy: pure data parallel — one sample per NeuronCore (8 cores).  Per-core
layout keeps channels on SBUF partitions (C=96 -> 96 partitions) and flattened
spatial positions on the free axis.  The grouped 3x3 convs become 9
PSUM-accumulated matmuls per output chunk, with per-offset block-diagonal
(96+1 x 96) weight matrices (built on host) against shifted views of a
zero-padded (58x58) SBUF image.  Row 96 of the padded image is all-ones so the
reference's `recon + 1e-12` rides along as an extra contraction row of the
offset-0 matmul.  Channel reductions (LayerNorm stats, NNMF renormalization)
are matmuls with a ones column; per-position scalars are broadcast back across
partitions with a (1 x 96) ones matmul.  Matmul operands are bitcast to
float32r (full-rate at N>=256 on TRN2).
"""

import numpy as np

C = 96
H = W = 56
NPIX = H * W          # 3136
HP = H + 2            # 58
PADPIX = HP * HP      # 3364
G, CG = 4, 24
NIT = 25
EPS = 1e-12
CH = 8                # image rows per chunk
NCHUNK = H // CH      # 7
CW = CH * W           # 448 positions per chunk
HID = 384

TRACE = False         # set True (e.g. from test.py) to collect NTFF exec time
LAST_RESULT = None    # BassKernelResults of the most recent run

_CACHED_NC = None


def _build_conv_mats(w_nnmf):
    """Per-offset lhsT matrices for both convs, packed (97, 9*96) f32."""
    w = np.abs(np.asarray(w_nnmf, np.float64))
    w = w / (w.sum(axis=(1, 2, 3), keepdims=True) + EPS)  # (96, 24, 3, 3)
    Wc = np.zeros((9, C + 1, C), np.float64)  # [k, i, o] = w[o, i_loc, dy, dx]
    Wr = np.zeros((9, C + 1, C), np.float64)  # [k, o, i] = w[o, i_loc, 2-dy, 2-dx]
    for dy in range(3):
        for dx in range(3):
            k = dy * 3 + dx
            blkc = w[:, :, dy, dx]          # (96 out, 24 in_local)
            blkr = w[:, :, 2 - dy, 2 - dx]  # (96 out, 24 in_local)
            for g in range(G):
                rows = slice(g * CG, (g + 1) * CG)
                Wc[k, rows, rows] = blkc[rows, :].T
                Wr[k, rows, rows] = blkr[rows, :]
    Wr[0, C, :] = EPS  # recon + EPS via the all-ones row of the padded image
    WcD = np.ascontiguousarray(Wc.transpose(1, 0, 2).reshape(C + 1, 9 * C), np.float32)
    WrD = np.ascontiguousarray(Wr.transpose(1, 0, 2).reshape(C + 1, 9 * C), np.float32)
    return WcD, WrD


def _build_bass(nit=NIT, gelu_mode="hw"):
    import concourse.bass as bass
    import concourse.bacc as bacc
    import concourse.mybir as mybir
    from concourse.tile import TileContext

    f32 = mybir.dt.float32
    bf16 = mybir.dt.bfloat16
    AF = mybir.ActivationFunctionType
    OP = mybir.AluOpType

    nc = bacc.Bacc(None, target_bir_lowering=False)

    x_d = nc.declare_dram_parameter("x", [C, NPIX], f32, isOutput=False)
    xb_d = nc.declare_dram_parameter("xbf", [C, NPIX], bf16, isOutput=False)
    wr_d = nc.declare_dram_parameter("wrecon", [C + 1, 9 * C], bf16, isOutput=False)
    wc_d = nc.declare_dram_parameter("wconv", [C + 1, 9 * C], bf16, isOutput=False)
    w1_d = nc.declare_dram_parameter("w1T", [C, HID], bf16, isOutput=False)
    b1_d = nc.declare_dram_parameter("b1", [HID, 1], f32, isOutput=False)
    w2_d = nc.declare_dram_parameter("w2T", [HID, C], bf16, isOutput=False)
    b2_d = nc.declare_dram_parameter("b2", [C, 1], f32, isOutput=False)
    ln1w_d = nc.declare_dram_parameter("ln1w", [C, 1], f32, isOutput=False)
    ln1b_d = nc.declare_dram_parameter("ln1b", [C, 1], f32, isOutput=False)
    ln2w_d = nc.declare_dram_parameter("ln2w", [C, 1], f32, isOutput=False)
    ln2b_d = nc.declare_dram_parameter("ln2b", [C, 1], f32, isOutput=False)
    out_d = nc.declare_dram_parameter("out", [C, NPIX], f32, isOutput=True)

    with TileContext(nc) as tc:
        with (
            tc.tile_pool(name="persist", bufs=1) as pp,
            tc.tile_pool(name="work", bufs=3) as wp,
            tc.tile_pool(name="small", bufs=3) as sp,
            tc.tile_pool(name="psconv", bufs=2, space="PSUM") as ps_conv,
            tc.tile_pool(name="psbig", bufs=2, space="PSUM") as ps_big,
            tc.tile_pool(name="pssum", bufs=2, space="PSUM") as ps_sum,
        ):
            # ---- persistent tiles ----
            xs = pp.tile([C, NPIX], f32, tag="xs")        # original x (residual)
            xb = pp.tile([C, NPIX], bf16, tag="xb")       # bf16 x for LN1 stats
            x2s = pp.tile([C, NPIX], f32, tag="x2s")      # x + attn residual
            xin = pp.tile([C, NPIX], f32, tag="xin")      # normalized relu(LN1(x))
            hpad = pp.tile([C + 1, PADPIX], bf16, tag="hpad")
            rpad = pp.tile([C + 1, PADPIX], bf16, tag="rpad")
            wr = pp.tile([C + 1, 9 * C], bf16, tag="wr")
            wc = pp.tile([C + 1, 9 * C], bf16, tag="wc")
            w1s = pp.tile([C, HID], bf16, tag="w1s")
            w2s = [pp.tile([128, C], bf16, tag=f"w2s{k}", name=f"w2s{k}")
                   for k in range(3)]
            b1s = pp.tile([128, 3], f32, tag="b1s")
            b2s = pp.tile([C, 1], f32, tag="b2s")
            ln1w = pp.tile([C, 1], f32, tag="ln1w")
            ln1b = pp.tile([C, 1], f32, tag="ln1b")
            ln2w = pp.tile([C, 1], f32, tag="ln2w")
            ln2b = pp.tile([C, 1], f32, tag="ln2b")
            ones_col = pp.tile([C, 1], bf16, tag="ones_col")   # channel-sum lhsT
            ones_row = pp.tile([1, C], bf16, tag="ones_row")   # broadcast lhsT
            eps6 = pp.tile([1, 1], f32, tag="eps6")            # 1e-6 (LN1)
            eps5 = pp.tile([1, 1], f32, tag="eps5")            # 1e-5 (LN2)

            # ---- load inputs ----
            nc.sync.dma_start(xs[:], x_d[:])
            nc.sync.dma_start(xb[:], xb_d[:])
            nc.sync.dma_start(wr[:], wr_d[:])
            nc.sync.dma_start(wc[:], wc_d[:])
            nc.sync.dma_start(w1s[:], w1_d[:])
            for k in range(3):
                nc.sync.dma_start(w2s[k][:], w2_d[k * 128:(k + 1) * 128, :])
            nc.sync.dma_start(b1s[:], b1_d[:].rearrange("(k p) one -> p (k one)", p=128))
            nc.sync.dma_start(b2s[:], b2_d[:])
            nc.sync.dma_start(ln1w[:], ln1w_d[:])
            nc.sync.dma_start(ln1b[:], ln1b_d[:])
            nc.sync.dma_start(ln2w[:], ln2w_d[:])
            nc.sync.dma_start(ln2b[:], ln2b_d[:])

            nc.vector.memset(ones_col[:], 1.0)
            nc.vector.memset(ones_row[:], 1.0)
            nc.vector.memset(eps6[:], 1e-6)
            nc.vector.memset(eps5[:], 1e-5)
            nc.vector.memset(hpad[:], 0.0)
            nc.vector.memset(rpad[:], 0.0)
            nc.vector.memset(hpad[C:C + 1, :], 1.0)
            nc.vector.memset(rpad[C:C + 1, :], 1.0)

            def pad3(t):
                return t[:].rearrange("p (h w) -> p h w", h=HP)

            def interior(t, y0, nrows):
                return pad3(t)[0:C, 1 + y0:1 + y0 + nrows, 1:1 + W]

            nc.vector.memset(interior(hpad, 0, H), 1.0 / C)

            def colsum(src_ap, eps):
                """sum over channels + eps -> reciprocal, returns (1, CW) bf16."""
                s = ps_sum.tile([1, CW], f32, tag="csum")
                nc.tensor.matmul(s[:], ones_col[:], src_ap)
                t = sp.tile([1, CW], f32, tag="cs_t")
                nc.vector.tensor_scalar_add(t[:], s[:], float(eps))
                rs = sp.tile([1, CW], bf16, tag="cs_r")
                with nc.allow_low_precision(reason="bf16 broadcast operand"):
                    nc.vector.reciprocal(rs[:], t[:])
                return rs

            def bcast(row_ap):
                """broadcast (1, CW) bf16 across C partitions -> PSUM (C, CW)."""
                b = ps_big.tile([C, CW], f32, tag="bcast")
                nc.tensor.matmul(b[:], ones_row[:], row_ap)
                return b

            def ln_stats(xc_f32, xc_bf16, eps_ap):
                """channel mean/istd of a (C, CW) chunk -> bf16 (1, CW) pair."""
                sq = wp.tile([C, CW], bf16, tag="ln_sq")
                nc.scalar.square(sq[:], xc_f32)
                s1 = ps_sum.tile([1, CW], f32, tag="csum")
                nc.tensor.matmul(s1[:], ones_col[:], xc_bf16)
                s2 = ps_sum.tile([1, CW], f32, tag="csum")
                nc.tensor.matmul(s2[:], ones_col[:], sq[:])
                u = sp.tile([1, CW], bf16, tag="ln_u")
                with nc.allow_low_precision(reason="bf16 broadcast operand"):
                    nc.vector.tensor_scalar_mul(u[:], s1[:], 1.0 / C)
                u2 = sp.tile([1, CW], f32, tag="ln_u2")
                nc.scalar.square(u2[:], u[:])
                var = sp.tile([1, CW], f32, tag="ln_var")
                nc.vector.scalar_tensor_tensor(
                    var[:], s2[:], 1.0 / C, u2[:], OP.mult, OP.subtract)
                sd = sp.tile([1, CW], f32, tag="ln_sd")
                nc.scalar.activation(sd[:], var[:], AF.Sqrt, bias=eps_ap)
                isd = sp.tile([1, CW], bf16, tag="ln_isd")
                with nc.allow_low_precision(reason="bf16 broadcast operand"):
                    nc.vector.reciprocal(isd[:], sd[:])
                return u, isd

            # ---- LN1 + relu + channel-normalize -> xin ----
            for c in range(NCHUNK):
                sl = slice(c * CW, (c + 1) * CW)
                xc = xs[:, sl]
                u, isd = ln_stats(xc, xb[:, sl], eps6[:, 0:1])
                ub = bcast(u[:])
                ib = bcast(isd[:])
                xm = wp.tile([C, CW], f32, tag="ln_xm")
                nc.vector.tensor_tensor(xm[:], xc, ub[:], OP.subtract)
                xn = wp.tile([C, CW], f32, tag="ln_xn")
                nc.vector.tensor_tensor(xn[:], xm[:], ib[:], OP.mult)
                rl = wp.tile([C, CW], bf16, tag="ln_rl")
                nc.scalar.activation(rl[:], xn[:], AF.Relu,
                                     bias=ln1b[:, 0:1], scale=ln1w[:, 0:1])
                rs = colsum(rl[:], EPS)
                sb = bcast(rs[:])
                nc.vector.tensor_tensor(xin[:, sl], rl[:], sb[:], OP.mult)

            # ---- NNMF iterations ----
            def nnmf_iteration(_iv=None):
                # Phase A: ratio = xin / (convT(h) + EPS) into padded buffer
                for c in range(NCHUNK):
                    y0 = c * CH
                    ps = ps_conv.tile([C, CW], f32, tag="conv")
                    for k in range(9):
                        dy, dx = k // 3, k % 3
                        view = pad3(hpad)[0:C + 1, y0 + dy:y0 + dy + CH, dx:dx + W]
                        nc.tensor.matmul(ps[:], wr[:, k * C:(k + 1) * C], view,
                                         start=(k == 0), stop=(k == 8))
                    rec = wp.tile([C, CW], f32, tag="rec")
                    nc.vector.reciprocal(rec[:], ps[:])
                    nc.vector.tensor_tensor(
                        interior(rpad, y0, CH),
                        xin[:, c * CW:(c + 1) * CW], rec[:], OP.mult)
                # Phase B: h = normalize(h * conv(ratio))
                for c in range(NCHUNK):
                    y0 = c * CH
                    ps = ps_conv.tile([C, CW], f32, tag="conv")
                    for k in range(9):
                        dy, dx = k // 3, k % 3
                        view = pad3(rpad)[0:C + 1, y0 + dy:y0 + dy + CH, dx:dx + W]
                        nc.tensor.matmul(ps[:], wc[:, k * C:(k + 1) * C], view,
                                         start=(k == 0), stop=(k == 8))
                    ht = wp.tile([C, CW], bf16, tag="ht")
                    nc.vector.tensor_tensor(ht[:], interior(hpad, y0, CH), ps[:],
                                            OP.mult)
                    rs = colsum(ht[:], EPS)
                    sb = bcast(rs[:])
                    nc.vector.tensor_tensor(interior(hpad, y0, CH), ht[:], sb[:],
                                            OP.mult)

            if nit > 0:
                with tc.For_i(0, nit, 1):
                    nnmf_iteration()

            # ---- x2 = x + h ----
            for c in range(NCHUNK):
                sl = slice(c * CW, (c + 1) * CW)
                nc.vector.tensor_tensor(x2s[:, sl], xs[:, sl],
                                        interior(hpad, c * CH, CH), OP.add)

            # ---- LN2 + MLP + residual -> out ----
            for c in range(NCHUNK):
                sl = slice(c * CW, (c + 1) * CW)
                xc = x2s[:, sl]
                x2b = wp.tile([C, CW], bf16, tag="x2b")
                nc.scalar.copy(x2b[:], xc)
                u, isd = ln_stats(xc, x2b[:], eps5[:, 0:1])
                ub = bcast(u[:])
                ib = bcast(isd[:])
                xm = wp.tile([C, CW], f32, tag="ln_xm")
                nc.vector.tensor_tensor(xm[:], xc, ub[:], OP.subtract)
                xn0 = wp.tile([C, CW], f32, tag="ln_xn")
                nc.vector.tensor_tensor(xn0[:], xm[:], ib[:], OP.mult)
                xn = wp.tile([C, CW], bf16, tag="ln_xw")
                nc.scalar.activation(xn[:], xn0[:], AF.Identity,
                                     bias=ln2b[:, 0:1], scale=ln2w[:, 0:1])
                ys = []
                for j in range(3):
                    p1 = ps_big.tile([128, CW], f32, tag="mlp1")
                    nc.tensor.matmul(p1[:], w1s[:, j * 128:(j + 1) * 128], xn[:])
                    y1 = wp.tile([128, CW], bf16, tag=f"mlp_y{j}", name=f"mlp_y{j}")
                    if gelu_mode == "hw":
                        nc.scalar.activation(y1[:], p1[:], AF.Gelu,
                                             bias=b1s[:, j:j + 1])
                    else:
                        # CoreSim fallback: sigmoid-GELU (Gelu not implemented
                        # in the simulator). Mirror must match.
                        pre = wp.tile([128, CW], f32, tag=f"mlp_p{j}",
                                      name=f"mlp_p{j}")
                        nc.scalar.activation(pre[:], p1[:], AF.Identity,
                                             bias=b1s[:, j:j + 1])
                        sg = wp.tile([128, CW], f32, tag=f"mlp_s{j}",
                                     name=f"mlp_s{j}")
                        nc.scalar.activation(sg[:], pre[:], AF.Sigmoid,
                                             scale=1.702)
                        nc.vector.tensor_tensor(y1[:], pre[:], sg[:], OP.mult)
                    ys.append(y1)
                p2 = ps_conv.tile([C, CW], f32, tag="conv")
                for k in range(3):
                    nc.tensor.matmul(p2[:], w2s[k][:], ys[k][:],
                                     start=(k == 0), stop=(k == 2))
                oc = wp.tile([C, CW], f32, tag="oc")
                nc.vector.scalar_tensor_tensor(
                    oc[:], p2[:], b2s[:, 0:1], xc, OP.add, OP.add)
                nc.sync.dma_start(out_d[:, sl], oc[:])

    return nc


def _prepare_maps(x, ln1_w, ln1_b, w_nnmf, ln2_w, ln2_b, w1, b1, w2, b2):
    import ml_dtypes
    bf16 = ml_dtypes.bfloat16
    WcD, WrD = _build_conv_mats(w_nnmf)
    f = lambda a: np.ascontiguousarray(np.asarray(a, np.float32))
    fb = lambda a: np.ascontiguousarray(np.asarray(a, np.float32).astype(bf16))
    shared = {
        "wrecon": fb(WrD),
        "wconv": fb(WcD),
        "w1T": fb(w1),
        "b1": f(b1).reshape(HID, 1),
        "w2T": fb(w2),
        "b2": f(b2).reshape(C, 1),
        "ln1w": f(ln1_w).reshape(C, 1),
        "ln1b": f(ln1_b).reshape(C, 1),
        "ln2w": f(ln2_w).reshape(C, 1),
        "ln2b": f(ln2_b).reshape(C, 1),
    }
    xs = np.asarray(x)
    return [dict(shared,
                 x=f(xs[i]).reshape(C, NPIX),
                 xbf=fb(xs[i]).reshape(C, NPIX))
            for i in range(xs.shape[0])]


def kernel(x, ln1_w, ln1_b, w_nnmf, ln2_w, ln2_b, w1, b1, w2, b2):
    global _CACHED_NC, LAST_RESULT
    from concourse.bass_utils import run_bass_kernel_spmd

    if _CACHED_NC is None:
        nc = _build_bass()
        nc.finalize()
        _CACHED_NC = nc
    nc = _CACHED_NC
    in_maps = _prepare_maps(x, ln1_w, ln1_b, w_nnmf, ln2_w, ln2_b, w1, b1, w2, b2)
    res = run_bass_kernel_spmd(nc, in_maps, core_ids=list(range(8)), trace=TRACE)
    LAST_RESULT = res
    out = np.stack([res.results[i]["out"].reshape(C, H, W) for i in range(8)])
    return out.astype(np.float32)


# revision 20
# speedup vs baseline: 1.9731x; 1.9731x over previous
"""Trainium2 Bass kernel for nn_Block_17978733101066.

ConvNeXt-style block: channels-first LayerNorm -> NNMF conv (25 multiplicative
updates with grouped 3x3 convs) residual branch, then channels-last LayerNorm +
MLP residual branch.  Input x: (8, 96, 56, 56) f32.

Strategy: pure data parallel — one sample per NeuronCore (8 cores).  Per-core
layout keeps channels on SBUF partitions (C=96 -> 96 partitions) and flattened
spatial positions on the free axis.  The grouped 3x3 convs become 9
PSUM-accumulated matmuls per output chunk, with per-offset block-diagonal
(96+1 x 96) weight matrices (built on host) against shifted views of a
zero-padded (58x58) SBUF image.  Row 96 of the padded image is all-ones so the
reference's `recon + 1e-12` rides along as an extra contraction row of the
offset-0 matmul.  Channel reductions (LayerNorm stats, NNMF renormalization)
are matmuls with a ones column; per-position scalars are broadcast back across
partitions with a (1 x 96) ones matmul.  Matmul operands are bitcast to
float32r (full-rate at N>=256 on TRN2).
"""

import numpy as np

C = 96
H = W = 56
NPIX = H * W          # 3136
HP = H + 2            # 58
PADPIX = HP * HP      # 3364
G, CG = 4, 24
NIT = 25
EPS = 1e-12
CH = 8                # image rows per chunk
NCHUNK = H // CH      # 7
CW = CH * W           # 448 positions per chunk
HID = 384

TRACE = False         # set True (e.g. from test.py) to collect NTFF exec time
LAST_RESULT = None    # BassKernelResults of the most recent run

_CACHED_NC = None


def _build_conv_mats(w_nnmf):
    """Per-offset lhsT matrices for both convs, packed (97, 9*96) f32."""
    w = np.abs(np.asarray(w_nnmf, np.float64))
    w = w / (w.sum(axis=(1, 2, 3), keepdims=True) + EPS)  # (96, 24, 3, 3)
    Wc = np.zeros((9, C + 1, C), np.float64)  # [k, i, o] = w[o, i_loc, dy, dx]
    Wr = np.zeros((9, C + 1, C), np.float64)  # [k, o, i] = w[o, i_loc, 2-dy, 2-dx]
    for dy in range(3):
        for dx in range(3):
            k = dy * 3 + dx
            blkc = w[:, :, dy, dx]          # (96 out, 24 in_local)
            blkr = w[:, :, 2 - dy, 2 - dx]  # (96 out, 24 in_local)
            for g in range(G):
                rows = slice(g * CG, (g + 1) * CG)
                Wc[k, rows, rows] = blkc[rows, :].T
                Wr[k, rows, rows] = blkr[rows, :]
    Wr[0, C, :] = EPS  # recon + EPS via the all-ones row of the padded image
    WcD = np.ascontiguousarray(Wc.transpose(1, 0, 2).reshape(C + 1, 9 * C), np.float32)
    WrD = np.ascontiguousarray(Wr.transpose(1, 0, 2).reshape(C + 1, 9 * C), np.float32)
    return WcD, WrD


def _build_bass(nit=NIT, gelu_mode="hw"):
    import concourse.bass as bass
    import concourse.bacc as bacc
    import concourse.mybir as mybir
    from concourse.tile import TileContext

    f32 = mybir.dt.float32
    bf16 = mybir.dt.bfloat16
    AF = mybir.ActivationFunctionType
    OP = mybir.AluOpType

    nc = bacc.Bacc(None, target_bir_lowering=False)

    x_d = nc.declare_dram_parameter("x", [C, NPIX], f32, isOutput=False)
    xb_d = nc.declare_dram_parameter("xbf", [C, NPIX], bf16, isOutput=False)
    wr_d = nc.declare_dram_parameter("wrecon", [C + 1, 9 * C], bf16, isOutput=False)
    wc_d = nc.declare_dram_parameter("wconv", [C + 1, 9 * C], bf16, isOutput=False)
    w1_d = nc.declare_dram_parameter("w1T", [C, HID], bf16, isOutput=False)
    b1_d = nc.declare_dram_parameter("b1", [HID, 1], f32, isOutput=False)
    w2_d = nc.declare_dram_parameter("w2T", [HID, C], bf16, isOutput=False)
    b2_d = nc.declare_dram_parameter("b2", [C, 1], f32, isOutput=False)
    ln1w_d = nc.declare_dram_parameter("ln1w", [C, 1], f32, isOutput=False)
    ln1b_d = nc.declare_dram_parameter("ln1b", [C, 1], f32, isOutput=False)
    ln2w_d = nc.declare_dram_parameter("ln2w", [C, 1], f32, isOutput=False)
    ln2b_d = nc.declare_dram_parameter("ln2b", [C, 1], f32, isOutput=False)
    out_d = nc.declare_dram_parameter("out", [C, NPIX], f32, isOutput=True)

    with TileContext(nc) as tc:
        with (
            tc.tile_pool(name="persist", bufs=1) as pp,
            tc.tile_pool(name="work", bufs=3) as wp,
            tc.tile_pool(name="small", bufs=3) as sp,
            tc.tile_pool(name="psconv", bufs=2, space="PSUM") as ps_conv,
            tc.tile_pool(name="psbig", bufs=2, space="PSUM") as ps_big,
            tc.tile_pool(name="pssum", bufs=2, space="PSUM") as ps_sum,
        ):
            # ---- persistent tiles ----
            xs = pp.tile([C, NPIX], f32, tag="xs")        # original x (residual)
            xb = pp.tile([C, NPIX], bf16, tag="xb")       # bf16 x for LN1 stats
            x2s = pp.tile([C, NPIX], f32, tag="x2s")      # x + attn residual
            xin = pp.tile([C, NPIX], f32, tag="xin")      # normalized relu(LN1(x))
            hpad = pp.tile([C + 1, PADPIX], bf16, tag="hpad")
            rpad = pp.tile([C + 1, PADPIX], bf16, tag="rpad")
            wr = pp.tile([C + 1, 9 * C], bf16, tag="wr")
            wc = pp.tile([C + 1, 9 * C], bf16, tag="wc")
            w1s = pp.tile([C, HID], bf16, tag="w1s")
            w2s = [pp.tile([128, C], bf16, tag=f"w2s{k}", name=f"w2s{k}")
                   for k in range(3)]
            b1s = pp.tile([128, 3], f32, tag="b1s")
            b2s = pp.tile([C, 1], f32, tag="b2s")
            ln1w = pp.tile([C, 1], f32, tag="ln1w")
            ln1b = pp.tile([C, 1], f32, tag="ln1b")
            ln2w = pp.tile([C, 1], f32, tag="ln2w")
            ln2b = pp.tile([C, 1], f32, tag="ln2b")
            ones_col = pp.tile([C, 1], bf16, tag="ones_col")   # channel-sum lhsT
            ones_row = pp.tile([1, C], bf16, tag="ones_row")   # broadcast lhsT
            eps6 = pp.tile([1, 1], f32, tag="eps6")            # 1e-6 (LN1)
            eps5 = pp.tile([1, 1], f32, tag="eps5")            # 1e-5 (LN2)

            # ---- load inputs ----
            nc.sync.dma_start(xs[:], x_d[:])
            nc.sync.dma_start(xb[:], xb_d[:])
            nc.sync.dma_start(wr[:], wr_d[:])
            nc.sync.dma_start(wc[:], wc_d[:])
            nc.sync.dma_start(w1s[:], w1_d[:])
            for k in range(3):
                nc.sync.dma_start(w2s[k][:], w2_d[k * 128:(k + 1) * 128, :])
            nc.sync.dma_start(b1s[:], b1_d[:].rearrange("(k p) one -> p (k one)", p=128))
            nc.sync.dma_start(b2s[:], b2_d[:])
            nc.sync.dma_start(ln1w[:], ln1w_d[:])
            nc.sync.dma_start(ln1b[:], ln1b_d[:])
            nc.sync.dma_start(ln2w[:], ln2w_d[:])
            nc.sync.dma_start(ln2b[:], ln2b_d[:])

            nc.vector.memset(ones_col[:], 1.0)
            nc.vector.memset(ones_row[:], 1.0)
            nc.vector.memset(eps6[:], 1e-6)
            nc.vector.memset(eps5[:], 1e-5)
            nc.vector.memset(hpad[:], 0.0)
            nc.vector.memset(rpad[:], 0.0)
            nc.vector.memset(hpad[C:C + 1, :], 1.0)
            nc.vector.memset(rpad[C:C + 1, :], 1.0)

            def pad3(t):
                return t[:].rearrange("p (h w) -> p h w", h=HP)

            def interior(t, y0, nrows):
                return pad3(t)[0:C, 1 + y0:1 + y0 + nrows, 1:1 + W]

            nc.vector.memset(interior(hpad, 0, H), 1.0 / C)

            def colsum(src_ap, eps):
                """sum over channels + eps -> reciprocal, returns (1, CW) bf16."""
                s = ps_sum.tile([1, CW], f32, tag="csum")
                nc.tensor.matmul(s[:], ones_col[:], src_ap)
                t = sp.tile([1, CW], f32, tag="cs_t")
                nc.vector.tensor_scalar_add(t[:], s[:], float(eps))
                rsf = sp.tile([1, CW], f32, tag="cs_rf")
                nc.vector.reciprocal_approx_fast(out=rsf[:], in_=t[:])
                rs = sp.tile([1, CW], bf16, tag="cs_r")
                nc.scalar.copy(rs[:], rsf[:])
                return rs

            def bcast(row_ap):
                """broadcast (1, CW) bf16 across C partitions -> PSUM (C, CW)."""
                b = ps_big.tile([C, CW], f32, tag="bcast")
                nc.tensor.matmul(b[:], ones_row[:], row_ap)
                return b

            def ln_stats(xc_f32, xc_bf16, eps_ap):
                """channel mean/istd of a (C, CW) chunk -> bf16 (1, CW) pair."""
                sq = wp.tile([C, CW], bf16, tag="ln_sq")
                nc.scalar.square(sq[:], xc_f32)
                s1 = ps_sum.tile([1, CW], f32, tag="csum")
                nc.tensor.matmul(s1[:], ones_col[:], xc_bf16)
                s2 = ps_sum.tile([1, CW], f32, tag="csum")
                nc.tensor.matmul(s2[:], ones_col[:], sq[:])
                u = sp.tile([1, CW], bf16, tag="ln_u")
                with nc.allow_low_precision(reason="bf16 broadcast operand"):
                    nc.vector.tensor_scalar_mul(u[:], s1[:], 1.0 / C)
                u2 = sp.tile([1, CW], f32, tag="ln_u2")
                nc.scalar.square(u2[:], u[:])
                var = sp.tile([1, CW], f32, tag="ln_var")
                nc.vector.scalar_tensor_tensor(
                    var[:], s2[:], 1.0 / C, u2[:], OP.mult, OP.subtract)
                sd = sp.tile([1, CW], f32, tag="ln_sd")
                nc.scalar.activation(sd[:], var[:], AF.Sqrt, bias=eps_ap)
                isdf = sp.tile([1, CW], f32, tag="ln_isdf")
                nc.vector.reciprocal_approx_fast(out=isdf[:], in_=sd[:])
                isd = sp.tile([1, CW], bf16, tag="ln_isd")
                nc.scalar.copy(isd[:], isdf[:])
                return u, isd

            # ---- LN1 + relu + channel-normalize -> xin ----
            for c in range(NCHUNK):
                sl = slice(c * CW, (c + 1) * CW)
                xc = xs[:, sl]
                u, isd = ln_stats(xc, xb[:, sl], eps6[:, 0:1])
                ub = bcast(u[:])
                ib = bcast(isd[:])
                xm = wp.tile([C, CW], f32, tag="ln_xm")
                nc.vector.tensor_tensor(xm[:], xc, ub[:], OP.subtract)
                xn = wp.tile([C, CW], f32, tag="ln_xn")
                nc.vector.tensor_tensor(xn[:], xm[:], ib[:], OP.mult)
                rl = wp.tile([C, CW], bf16, tag="ln_rl")
                nc.scalar.activation(rl[:], xn[:], AF.Relu,
                                     bias=ln1b[:, 0:1], scale=ln1w[:, 0:1])
                rs = colsum(rl[:], EPS)
                sb = bcast(rs[:])
                nc.vector.tensor_tensor(xin[:, sl], rl[:], sb[:], OP.mult)

            # ---- NNMF iterations ----
            def nnmf_iteration(_iv=None):
                # Phase A: ratio = xin / (convT(h) + EPS) into padded buffer
                for c in range(NCHUNK):
                    y0 = c * CH
                    ps = ps_conv.tile([C, CW], f32, tag="conv")
                    for k in range(9):
                        dy, dx = k // 3, k % 3
                        view = pad3(hpad)[0:C + 1, y0 + dy:y0 + dy + CH, dx:dx + W]
                        nc.tensor.matmul(ps[:], wr[:, k * C:(k + 1) * C], view,
                                         start=(k == 0), stop=(k == 8))
                    rec = wp.tile([C, CW], f32, tag="rec")
                    nc.vector.reciprocal_approx_fast(out=rec[:], in_=ps[:])
                    nc.vector.tensor_tensor(
                        interior(rpad, y0, CH),
                        xin[:, c * CW:(c + 1) * CW], rec[:], OP.mult)
                # Phase B: h = normalize(h * conv(ratio))
                for c in range(NCHUNK):
                    y0 = c * CH
                    ps = ps_conv.tile([C, CW], f32, tag="conv")
                    for k in range(9):
                        dy, dx = k // 3, k % 3
                        view = pad3(rpad)[0:C + 1, y0 + dy:y0 + dy + CH, dx:dx + W]
                        nc.tensor.matmul(ps[:], wc[:, k * C:(k + 1) * C], view,
                                         start=(k == 0), stop=(k == 8))
                    ht = wp.tile([C, CW], bf16, tag="ht")
                    nc.vector.tensor_tensor(ht[:], interior(hpad, y0, CH), ps[:],
                                            OP.mult)
                    rs = colsum(ht[:], EPS)
                    sb = bcast(rs[:])
                    nc.vector.tensor_tensor(interior(hpad, y0, CH), ht[:], sb[:],
                                            OP.mult)

            if nit > 0:
                tc.For_i_unrolled(0, nit, 1, nnmf_iteration, max_unroll=5)

            # ---- x2 = x + h ----
            for c in range(NCHUNK):
                sl = slice(c * CW, (c + 1) * CW)
                nc.vector.tensor_tensor(x2s[:, sl], xs[:, sl],
                                        interior(hpad, c * CH, CH), OP.add)

            # ---- LN2 + MLP + residual -> out ----
            for c in range(NCHUNK):
                sl = slice(c * CW, (c + 1) * CW)
                xc = x2s[:, sl]
                x2b = wp.tile([C, CW], bf16, tag="x2b")
                nc.scalar.copy(x2b[:], xc)
                u, isd = ln_stats(xc, x2b[:], eps5[:, 0:1])
                ub = bcast(u[:])
                ib = bcast(isd[:])
                xm = wp.tile([C, CW], f32, tag="ln_xm")
                nc.vector.tensor_tensor(xm[:], xc, ub[:], OP.subtract)
                xn0 = wp.tile([C, CW], f32, tag="ln_xn")
                nc.vector.tensor_tensor(xn0[:], xm[:], ib[:], OP.mult)
                xn = wp.tile([C, CW], bf16, tag="ln_xw")
                nc.scalar.activation(xn[:], xn0[:], AF.Identity,
                                     bias=ln2b[:, 0:1], scale=ln2w[:, 0:1])
                ys = []
                for j in range(3):
                    p1 = ps_big.tile([128, CW], f32, tag="mlp1")
                    nc.tensor.matmul(p1[:], w1s[:, j * 128:(j + 1) * 128], xn[:])
                    y1 = wp.tile([128, CW], bf16, tag=f"mlp_y{j}", name=f"mlp_y{j}")
                    if gelu_mode == "hw":
                        nc.scalar.activation(y1[:], p1[:], AF.Gelu,
                                             bias=b1s[:, j:j + 1])
                    else:
                        # CoreSim fallback: sigmoid-GELU (Gelu not implemented
                        # in the simulator). Mirror must match.
                        pre = wp.tile([128, CW], f32, tag=f"mlp_p{j}",
                                      name=f"mlp_p{j}")
                        nc.scalar.activation(pre[:], p1[:], AF.Identity,
                                             bias=b1s[:, j:j + 1])
                        sg = wp.tile([128, CW], f32, tag=f"mlp_s{j}",
                                     name=f"mlp_s{j}")
                        nc.scalar.activation(sg[:], pre[:], AF.Sigmoid,
                                             scale=1.702)
                        nc.vector.tensor_tensor(y1[:], pre[:], sg[:], OP.mult)
                    ys.append(y1)
                p2 = ps_conv.tile([C, CW], f32, tag="conv")
                for k in range(3):
                    nc.tensor.matmul(p2[:], w2s[k][:], ys[k][:],
                                     start=(k == 0), stop=(k == 2))
                oc = wp.tile([C, CW], f32, tag="oc")
                nc.vector.scalar_tensor_tensor(
                    oc[:], p2[:], b2s[:, 0:1], xc, OP.add, OP.add)
                nc.sync.dma_start(out_d[:, sl], oc[:])

    return nc


def _prepare_maps(x, ln1_w, ln1_b, w_nnmf, ln2_w, ln2_b, w1, b1, w2, b2):
    import ml_dtypes
    bf16 = ml_dtypes.bfloat16
    WcD, WrD = _build_conv_mats(w_nnmf)
    f = lambda a: np.ascontiguousarray(np.asarray(a, np.float32))
    fb = lambda a: np.ascontiguousarray(np.asarray(a, np.float32).astype(bf16))
    shared = {
        "wrecon": fb(WrD),
        "wconv": fb(WcD),
        "w1T": fb(w1),
        "b1": f(b1).reshape(HID, 1),
        "w2T": fb(w2),
        "b2": f(b2).reshape(C, 1),
        "ln1w": f(ln1_w).reshape(C, 1),
        "ln1b": f(ln1_b).reshape(C, 1),
        "ln2w": f(ln2_w).reshape(C, 1),
        "ln2b": f(ln2_b).reshape(C, 1),
    }
    xs = np.asarray(x)
    return [dict(shared,
                 x=f(xs[i]).reshape(C, NPIX),
                 xbf=fb(xs[i]).reshape(C, NPIX))
            for i in range(xs.shape[0])]


def kernel(x, ln1_w, ln1_b, w_nnmf, ln2_w, ln2_b, w1, b1, w2, b2):
    global _CACHED_NC, LAST_RESULT
    from concourse.bass_utils import run_bass_kernel_spmd

    if _CACHED_NC is None:
        nc = _build_bass()
        nc.finalize()
        _CACHED_NC = nc
    nc = _CACHED_NC
    in_maps = _prepare_maps(x, ln1_w, ln1_b, w_nnmf, ln2_w, ln2_b, w1, b1, w2, b2)
    res = run_bass_kernel_spmd(nc, in_maps, core_ids=list(range(8)), trace=TRACE)
    LAST_RESULT = res
    out = np.stack([res.results[i]["out"].reshape(C, H, W) for i in range(8)])
    return out.astype(np.float32)


# revision 27
# speedup vs baseline: 2.0774x; 1.0529x over previous
"""Trainium2 Bass kernel for nn_Block_17978733101066.

ConvNeXt-style block: channels-first LayerNorm -> NNMF conv (25 multiplicative
updates with grouped 3x3 convs) residual branch, then channels-last LayerNorm +
MLP residual branch.  Input x: (8, 96, 56, 56) f32.

Strategy: pure data parallel — one sample per NeuronCore (8 cores).  Per-core
layout keeps channels on SBUF partitions (C=96 -> 96 partitions) and flattened
spatial positions on the free axis.  The grouped 3x3 convs become 9
PSUM-accumulated matmuls per output chunk, with per-offset block-diagonal
(96+1 x 96) weight matrices (built on host) against shifted views of a
zero-padded (58x58) SBUF image.  Row 96 of the padded image is all-ones so the
reference's `recon + 1e-12` rides along as an extra contraction row of the
offset-0 matmul.  Channel reductions (LayerNorm stats, NNMF renormalization)
are matmuls with a ones column; per-position scalars are broadcast back across
partitions with a (1 x 96) ones matmul.  Matmul operands are bitcast to
float32r (full-rate at N>=256 on TRN2).
"""

import numpy as np

C = 96
H = W = 56
NPIX = H * W          # 3136
HP = H + 2            # 58
PADPIX = HP * HP      # 3364
G, CG = 4, 24
NIT = 25
EPS = 1e-12
CH = 8                # image rows per chunk
NCHUNK = H // CH      # 7
CW = CH * W           # 448 positions per chunk
HID = 384

TRACE = False         # set True (e.g. from test.py) to collect NTFF exec time
LAST_RESULT = None    # BassKernelResults of the most recent run

_CACHED_NC = None


def _build_conv_mats(w_nnmf):
    """Per-offset lhsT matrices for both convs, packed (97, 9*96) f32."""
    w = np.abs(np.asarray(w_nnmf, np.float64))
    w = w / (w.sum(axis=(1, 2, 3), keepdims=True) + EPS)  # (96, 24, 3, 3)
    Wc = np.zeros((9, C + 1, C), np.float64)  # [k, i, o] = w[o, i_loc, dy, dx]
    Wr = np.zeros((9, C + 1, C), np.float64)  # [k, o, i] = w[o, i_loc, 2-dy, 2-dx]
    for dy in range(3):
        for dx in range(3):
            k = dy * 3 + dx
            blkc = w[:, :, dy, dx]          # (96 out, 24 in_local)
            blkr = w[:, :, 2 - dy, 2 - dx]  # (96 out, 24 in_local)
            for g in range(G):
                rows = slice(g * CG, (g + 1) * CG)
                Wc[k, rows, rows] = blkc[rows, :].T
                Wr[k, rows, rows] = blkr[rows, :]
    Wr[0, C, :] = EPS  # recon + EPS via the all-ones row of the padded image
    WcD = np.ascontiguousarray(Wc.transpose(1, 0, 2).reshape(C + 1, 9 * C), np.float32)
    WrD = np.ascontiguousarray(Wr.transpose(1, 0, 2).reshape(C + 1, 9 * C), np.float32)
    return WcD, WrD


def _build_bass(nit=NIT, gelu_mode="hw"):
    import concourse.bass as bass
    import concourse.bacc as bacc
    import concourse.mybir as mybir
    from concourse.tile import TileContext

    f32 = mybir.dt.float32
    bf16 = mybir.dt.bfloat16
    AF = mybir.ActivationFunctionType
    OP = mybir.AluOpType

    nc = bacc.Bacc(None, target_bir_lowering=False)

    x_d = nc.declare_dram_parameter("x", [C, NPIX], f32, isOutput=False)
    xb_d = nc.declare_dram_parameter("xbf", [C, NPIX], bf16, isOutput=False)
    wr_d = nc.declare_dram_parameter("wrecon", [C + 1, 9 * C], bf16, isOutput=False)
    wc_d = nc.declare_dram_parameter("wconv", [C + 1, 9 * C], bf16, isOutput=False)
    w1_d = nc.declare_dram_parameter("w1T", [C, HID], bf16, isOutput=False)
    b1_d = nc.declare_dram_parameter("b1", [HID, 1], f32, isOutput=False)
    w2_d = nc.declare_dram_parameter("w2T", [HID, C], bf16, isOutput=False)
    b2_d = nc.declare_dram_parameter("b2", [C, 1], f32, isOutput=False)
    ln1w_d = nc.declare_dram_parameter("ln1w", [C, 1], f32, isOutput=False)
    ln1b_d = nc.declare_dram_parameter("ln1b", [C, 1], f32, isOutput=False)
    ln2w_d = nc.declare_dram_parameter("ln2w", [C, 1], f32, isOutput=False)
    ln2b_d = nc.declare_dram_parameter("ln2b", [C, 1], f32, isOutput=False)
    out_d = nc.declare_dram_parameter("out", [C, NPIX], f32, isOutput=True)

    with TileContext(nc) as tc:
        with (
            tc.tile_pool(name="persist", bufs=1) as pp,
            tc.tile_pool(name="work", bufs=3) as wp,
            tc.tile_pool(name="small", bufs=3) as sp,
            tc.tile_pool(name="psconv", bufs=2, space="PSUM") as ps_conv,
            tc.tile_pool(name="psbig", bufs=2, space="PSUM") as ps_big,
            tc.tile_pool(name="pssum", bufs=2, space="PSUM") as ps_sum,
        ):
            # ---- persistent tiles ----
            xs = pp.tile([C, NPIX], f32, tag="xs")        # original x (residual)
            xb = pp.tile([C, NPIX], bf16, tag="xb")       # bf16 x for LN1 stats
            x2s = pp.tile([C, NPIX], f32, tag="x2s")      # x + attn residual
            xin = pp.tile([C, NPIX], f32, tag="xin")      # normalized relu(LN1(x))
            hpad = pp.tile([C + 1, PADPIX], bf16, tag="hpad")
            rpad = pp.tile([C + 1, PADPIX], bf16, tag="rpad")
            wr = pp.tile([C + 1, 9 * C], bf16, tag="wr")
            wc = pp.tile([C + 1, 9 * C], bf16, tag="wc")
            w1s = pp.tile([C, HID], bf16, tag="w1s")
            w2s = [pp.tile([128, C], bf16, tag=f"w2s{k}", name=f"w2s{k}")
                   for k in range(3)]
            b1s = pp.tile([128, 3], f32, tag="b1s")
            b2s = pp.tile([C, 1], f32, tag="b2s")
            ln1w = pp.tile([C, 1], f32, tag="ln1w")
            ln1b = pp.tile([C, 1], f32, tag="ln1b")
            ln2w = pp.tile([C, 1], f32, tag="ln2w")
            ln2b = pp.tile([C, 1], f32, tag="ln2b")
            ones_col = pp.tile([C, 1], bf16, tag="ones_col")   # channel-sum lhsT
            ones_row = pp.tile([1, C], bf16, tag="ones_row")   # broadcast lhsT
            eps6 = pp.tile([1, 1], f32, tag="eps6")            # 1e-6 (LN1)
            eps5 = pp.tile([1, 1], f32, tag="eps5")            # 1e-5 (LN2)
            eps12 = pp.tile([1, 1], f32, tag="eps12")          # 1e-12 (colsum)

            # ---- load inputs ----
            nc.sync.dma_start(xs[:], x_d[:])
            nc.sync.dma_start(xb[:], xb_d[:])
            nc.sync.dma_start(wr[:], wr_d[:])
            nc.sync.dma_start(wc[:], wc_d[:])
            nc.sync.dma_start(w1s[:], w1_d[:])
            for k in range(3):
                nc.sync.dma_start(w2s[k][:], w2_d[k * 128:(k + 1) * 128, :])
            nc.sync.dma_start(b1s[:], b1_d[:].rearrange("(k p) one -> p (k one)", p=128))
            nc.sync.dma_start(b2s[:], b2_d[:])
            nc.sync.dma_start(ln1w[:], ln1w_d[:])
            nc.sync.dma_start(ln1b[:], ln1b_d[:])
            nc.sync.dma_start(ln2w[:], ln2w_d[:])
            nc.sync.dma_start(ln2b[:], ln2b_d[:])

            nc.vector.memset(ones_col[:], 1.0)
            nc.vector.memset(ones_row[:], 1.0)
            nc.vector.memset(eps6[:], 1e-6)
            nc.vector.memset(eps5[:], 1e-5)
            nc.vector.memset(eps12[:], 1e-12)
            nc.vector.memset(hpad[:], 0.0)
            nc.vector.memset(rpad[:], 0.0)
            nc.vector.memset(hpad[C:C + 1, :], 1.0)
            nc.vector.memset(rpad[C:C + 1, :], 1.0)

            def pad3(t):
                return t[:].rearrange("p (h w) -> p h w", h=HP)

            def interior(t, y0, nrows):
                return pad3(t)[0:C, 1 + y0:1 + y0 + nrows, 1:1 + W]

            nc.vector.memset(interior(hpad, 0, H), 1.0 / C)

            def colsum(src_ap, eps_tile):
                """sum over channels + eps -> reciprocal, returns (1, CW) bf16."""
                s = ps_sum.tile([1, CW], f32, tag="csum")
                nc.tensor.matmul(s[:], ones_col[:], src_ap)
                t = sp.tile([1, CW], f32, tag="cs_t")
                nc.scalar.activation(t[:], s[:], AF.Identity, bias=eps_tile[:, 0:1])
                rsf = sp.tile([1, CW], f32, tag="cs_rf")
                nc.vector.reciprocal_approx_fast(out=rsf[:], in_=t[:])
                rs = sp.tile([1, CW], bf16, tag="cs_r")
                nc.scalar.copy(rs[:], rsf[:])
                return rs

            def bcast(row_ap):
                """broadcast (1, CW) bf16 across C partitions -> PSUM (C, CW)."""
                b = ps_big.tile([C, CW], f32, tag="bcast")
                nc.tensor.matmul(b[:], ones_row[:], row_ap)
                return b

            def ln_stats(xc_f32, xc_bf16, eps_ap):
                """channel mean/istd of a (C, CW) chunk -> bf16 (1, CW) pair."""
                sq = wp.tile([C, CW], bf16, tag="ln_sq")
                nc.scalar.square(sq[:], xc_f32)
                s1 = ps_sum.tile([1, CW], f32, tag="csum")
                nc.tensor.matmul(s1[:], ones_col[:], xc_bf16)
                s2 = ps_sum.tile([1, CW], f32, tag="csum")
                nc.tensor.matmul(s2[:], ones_col[:], sq[:])
                u = sp.tile([1, CW], bf16, tag="ln_u")
                with nc.allow_low_precision(reason="bf16 broadcast operand"):
                    nc.vector.tensor_scalar_mul(u[:], s1[:], 1.0 / C)
                u2 = sp.tile([1, CW], f32, tag="ln_u2")
                nc.scalar.square(u2[:], u[:])
                var = sp.tile([1, CW], f32, tag="ln_var")
                nc.vector.scalar_tensor_tensor(
                    var[:], s2[:], 1.0 / C, u2[:], OP.mult, OP.subtract)
                sd = sp.tile([1, CW], f32, tag="ln_sd")
                nc.scalar.activation(sd[:], var[:], AF.Sqrt, bias=eps_ap)
                isdf = sp.tile([1, CW], f32, tag="ln_isdf")
                nc.vector.reciprocal_approx_fast(out=isdf[:], in_=sd[:])
                isd = sp.tile([1, CW], bf16, tag="ln_isd")
                nc.scalar.copy(isd[:], isdf[:])
                return u, isd

            # ---- LN1 + relu + channel-normalize -> xin ----
            for c in range(NCHUNK):
                sl = slice(c * CW, (c + 1) * CW)
                xc = xs[:, sl]
                u, isd = ln_stats(xc, xb[:, sl], eps6[:, 0:1])
                ub = bcast(u[:])
                ib = bcast(isd[:])
                xm = wp.tile([C, CW], f32, tag="ln_xm")
                nc.vector.tensor_tensor(xm[:], xc, ub[:], OP.subtract)
                xn = wp.tile([C, CW], f32, tag="ln_xn")
                nc.vector.tensor_tensor(xn[:], xm[:], ib[:], OP.mult)
                rl = wp.tile([C, CW], bf16, tag="ln_rl")
                nc.scalar.activation(rl[:], xn[:], AF.Relu,
                                     bias=ln1b[:, 0:1], scale=ln1w[:, 0:1])
                rs = colsum(rl[:], eps12)
                sb = bcast(rs[:])
                nc.vector.tensor_tensor(xin[:, sl], rl[:], sb[:], OP.mult)

            # ---- NNMF iterations ----
            def nnmf_iteration(_iv=None):
                # Phase A: ratio = xin / (convT(h) + EPS) into padded buffer
                for c in range(NCHUNK):
                    y0 = c * CH
                    ps = ps_conv.tile([C, CW], f32, tag="conv")
                    for k in range(9):
                        dy, dx = k // 3, k % 3
                        view = pad3(hpad)[0:C + 1, y0 + dy:y0 + dy + CH, dx:dx + W]
                        nc.tensor.matmul(ps[:], wr[:, k * C:(k + 1) * C], view,
                                         start=(k == 0), stop=(k == 8))
                    rec = wp.tile([C, CW], f32, tag="rec")
                    nc.vector.reciprocal_approx_fast(out=rec[:], in_=ps[:])
                    # gpsimd (idle in the loop) handles the SBUF-only multiply
                    nc.gpsimd.tensor_tensor(
                        interior(rpad, y0, CH),
                        xin[:, c * CW:(c + 1) * CW], rec[:], OP.mult)
                # Phase B: h = normalize(h * conv(ratio))
                for c in range(NCHUNK):
                    y0 = c * CH
                    ps = ps_conv.tile([C, CW], f32, tag="conv")
                    for k in range(9):
                        dy, dx = k // 3, k % 3
                        view = pad3(rpad)[0:C + 1, y0 + dy:y0 + dy + CH, dx:dx + W]
                        nc.tensor.matmul(ps[:], wc[:, k * C:(k + 1) * C], view,
                                         start=(k == 0), stop=(k == 8))
                    ht = wp.tile([C, CW], bf16, tag="ht")
                    nc.vector.tensor_tensor(ht[:], interior(hpad, y0, CH), ps[:],
                                            OP.mult)
                    rs = colsum(ht[:], eps12)
                    sb = bcast(rs[:])
                    nc.vector.tensor_tensor(interior(hpad, y0, CH), ht[:], sb[:],
                                            OP.mult)

            for _ in range(nit):
                nnmf_iteration()

            # ---- x2 = x + h ----
            for c in range(NCHUNK):
                sl = slice(c * CW, (c + 1) * CW)
                nc.vector.tensor_tensor(x2s[:, sl], xs[:, sl],
                                        interior(hpad, c * CH, CH), OP.add)

            # ---- LN2 + MLP + residual -> out ----
            for c in range(NCHUNK):
                sl = slice(c * CW, (c + 1) * CW)
                xc = x2s[:, sl]
                x2b = wp.tile([C, CW], bf16, tag="x2b")
                nc.scalar.copy(x2b[:], xc)
                u, isd = ln_stats(xc, x2b[:], eps5[:, 0:1])
                ub = bcast(u[:])
                ib = bcast(isd[:])
                xm = wp.tile([C, CW], f32, tag="ln_xm")
                nc.vector.tensor_tensor(xm[:], xc, ub[:], OP.subtract)
                xn0 = wp.tile([C, CW], f32, tag="ln_xn")
                nc.vector.tensor_tensor(xn0[:], xm[:], ib[:], OP.mult)
                xn = wp.tile([C, CW], bf16, tag="ln_xw")
                nc.scalar.activation(xn[:], xn0[:], AF.Identity,
                                     bias=ln2b[:, 0:1], scale=ln2w[:, 0:1])
                ys = []
                for j in range(3):
                    p1 = ps_big.tile([128, CW], f32, tag="mlp1")
                    nc.tensor.matmul(p1[:], w1s[:, j * 128:(j + 1) * 128], xn[:])
                    y1 = wp.tile([128, CW], bf16, tag=f"mlp_y{j}", name=f"mlp_y{j}")
                    if gelu_mode == "hw":
                        nc.scalar.activation(y1[:], p1[:], AF.Gelu,
                                             bias=b1s[:, j:j + 1])
                    else:
                        # CoreSim fallback: sigmoid-GELU (Gelu not implemented
                        # in the simulator). Mirror must match.
                        pre = wp.tile([128, CW], f32, tag=f"mlp_p{j}",
                                      name=f"mlp_p{j}")
                        nc.scalar.activation(pre[:], p1[:], AF.Identity,
                                             bias=b1s[:, j:j + 1])
                        sg = wp.tile([128, CW], f32, tag=f"mlp_s{j}",
                                     name=f"mlp_s{j}")
                        nc.scalar.activation(sg[:], pre[:], AF.Sigmoid,
                                             scale=1.702)
                        nc.vector.tensor_tensor(y1[:], pre[:], sg[:], OP.mult)
                    ys.append(y1)
                p2 = ps_conv.tile([C, CW], f32, tag="conv")
                for k in range(3):
                    nc.tensor.matmul(p2[:], w2s[k][:], ys[k][:],
                                     start=(k == 0), stop=(k == 2))
                oc = wp.tile([C, CW], f32, tag="oc")
                nc.vector.scalar_tensor_tensor(
                    oc[:], p2[:], b2s[:, 0:1], xc, OP.add, OP.add)
                nc.sync.dma_start(out_d[:, sl], oc[:])

    return nc


def _prepare_maps(x, ln1_w, ln1_b, w_nnmf, ln2_w, ln2_b, w1, b1, w2, b2):
    import ml_dtypes
    bf16 = ml_dtypes.bfloat16
    WcD, WrD = _build_conv_mats(w_nnmf)
    f = lambda a: np.ascontiguousarray(np.asarray(a, np.float32))
    fb = lambda a: np.ascontiguousarray(np.asarray(a, np.float32).astype(bf16))
    shared = {
        "wrecon": fb(WrD),
        "wconv": fb(WcD),
        "w1T": fb(w1),
        "b1": f(b1).reshape(HID, 1),
        "w2T": fb(w2),
        "b2": f(b2).reshape(C, 1),
        "ln1w": f(ln1_w).reshape(C, 1),
        "ln1b": f(ln1_b).reshape(C, 1),
        "ln2w": f(ln2_w).reshape(C, 1),
        "ln2b": f(ln2_b).reshape(C, 1),
    }
    xs = np.asarray(x)
    return [dict(shared,
                 x=f(xs[i]).reshape(C, NPIX),
                 xbf=fb(xs[i]).reshape(C, NPIX))
            for i in range(xs.shape[0])]


def kernel(x, ln1_w, ln1_b, w_nnmf, ln2_w, ln2_b, w1, b1, w2, b2):
    global _CACHED_NC, LAST_RESULT
    from concourse.bass_utils import run_bass_kernel_spmd

    if _CACHED_NC is None:
        nc = _build_bass()
        nc.finalize()
        _CACHED_NC = nc
    nc = _CACHED_NC
    in_maps = _prepare_maps(x, ln1_w, ln1_b, w_nnmf, ln2_w, ln2_b, w1, b1, w2, b2)
    res = run_bass_kernel_spmd(nc, in_maps, core_ids=list(range(8)), trace=TRACE)
    LAST_RESULT = res
    out = np.stack([res.results[i]["out"].reshape(C, H, W) for i in range(8)])
    return out.astype(np.float32)
